# revision 10
# baseline (speedup 1.0000x reference)
"""MoE layer v4: token-data-parallel, bf16 FFN, transpose-gather.

Per core (1024 tokens): router computes top-2 combine weights in fp32
(x-chunk-stationary matmuls on quarter-T x tiles); per-expert exclusive
ranks via a running-cumsum of triangular/ones matmuls. All-expert prep
runs immediately after the router (during the shared expert): per expert
a 0/1 slot-selection matrix in fp16 and ONE flipped matmul chain ([128,2]
stationary: tokid+1 | gating) yields slot->(tokid+1) and slot gatings on
PSUM partitions 0-1; ids get -1 so empty slots hold -1 (trailing) and the
scatter skips them. ids/gatings roundtrip DRAM to become gather indices
(idx layout [128, C/16], single strided DMA) and a [128,3] gating layout.
dma_gather(transpose=True) pulls 384 selected token rows from the bf16
token table directly as X_e^T; gathers run 2-3 experts ahead. SwiGLU runs
in bf16 on a per-expert compute width NE_W[e] = max-load + margin <= 320
slots with packed 938-wide weights; outputs are scaled by slot gatings
and dma_scatter_add'ed per capacity chunk (128,128,NE_W-256) into the
bf16 output, skipping empty slots. The shared expert runs dense in bf16
over two 512-token chunks, writing output rows directly. Weight / x / gw
loads are batched into few 3D-AP DMAs to minimize serialized per-
instruction DMA issue cost on the sync engine.
"""

import os
import numpy as np
from contextlib import ExitStack

_DBG = 4  # debug build levels retained for bisection; 4 = full kernel

import ml_dtypes

import concourse.bass as bass
import concourse.mybir as mybir
import concourse.tile as tile
from concourse import bacc
from concourse.bass_utils import run_bass_kernel_spmd

B, S, D = 4, 2048, 1024
E = 8
I = 938
GU2 = 2 * I          # 1876 packed gate|up
NE = E + 1
N_CORES = 8
T = (B * S) // N_CORES   # 1024 tokens/core
C = 384                  # gather slot space (transpose gather needs %128)
CB = C // 128

# per-expert compute width: max load over cores (282 global) + margin
NE_W = [296, 292, 296, 304, 300, 296, 296, 296]

P = 128
KD = D // P              # 8 d-chunks
MT = T // P              # 8 token chunks
MI = (I + P - 1) // P    # 8 i-chunks (last = 42 rows)
MW_LAST = I - (MI - 1) * P   # 42
IP = MI * P              # 1024 (wdn dram padded rows)

F32 = mybir.dt.float32
F32R = mybir.dt.float32r
BF16 = mybir.dt.bfloat16
FP16 = mybir.dt.float16
I16 = mybir.dt.int16
AF = mybir.ActivationFunctionType
OP = mybir.AluOpType
AX = mybir.AxisListType

SH_CHUNKS = [(0, 512), (512, 512)]   # shared expert token chunks
GATHER_AHEAD = 3                     # experts of gather prefetch


def build_moe():
    nc = bacc.Bacc("TRN2", target_bir_lowering=False, debug=False,
                   enable_asserts=True, num_devices=N_CORES)
    xT = nc.dram_tensor("xT", [D, T], F32R, kind="ExternalInput")
    xTbf = nc.dram_tensor("xTbf", [D, T], BF16, kind="ExternalInput")
    xRbf = nc.dram_tensor("xRbf", [T, D], BF16, kind="ExternalInput")
    gwT = nc.dram_tensor("gwT", [D, E], F32, kind="ExternalInput")
    wgu = nc.dram_tensor("wgu", [NE, D, GU2], BF16, kind="ExternalInput")
    wdn = nc.dram_tensor("wdn", [NE, IP, D], BF16, kind="ExternalInput")
    triu = nc.dram_tensor("triu", [P, P], F32R, kind="ExternalInput")
    ones = nc.dram_tensor("ones", [P, P], F32R, kind="ExternalInput")
    iotaC = nc.dram_tensor("iotaC", [P, C], F32, kind="ExternalInput")
    iotaT2 = nc.dram_tensor("iotaT2", [P, MT], FP16, kind="ExternalInput")
    out = nc.dram_tensor("out", [T, D], BF16, kind="ExternalOutput")

    with tile.TileContext(nc) as tc, ExitStack() as ctx:
        xts_pool = ctx.enter_context(tc.tile_pool(name="xts", bufs=2))
        gwc_pool = ctx.enter_context(tc.tile_pool(name="gwc", bufs=KD))
        xtbf_pool = ctx.enter_context(tc.tile_pool(name="xtbf", bufs=KD))
        wgu_pool = ctx.enter_context(tc.tile_pool(name="wgu", bufs=16))
        wdn_pool = ctx.enter_context(tc.tile_pool(name="wdn", bufs=15))
        xet_pool = ctx.enter_context(tc.tile_pool(name="xet", bufs=3))
        at_pool = ctx.enter_context(tc.tile_pool(name="at", bufs=10))
        st_pool = ctx.enter_context(tc.tile_pool(name="st", bufs=3))
        ot_pool = ctx.enter_context(tc.tile_pool(name="ot", bufs=4))
        y_pool = ctx.enter_context(tc.tile_pool(name="y", bufs=4))
        sel_pool = ctx.enter_context(tc.tile_pool(name="sel", bufs=10))
        rt_pool = ctx.enter_context(tc.tile_pool(name="rt", bufs=4))
        rhs2_pool = ctx.enter_context(tc.tile_pool(name="rhs2", bufs=8))
        idx_pool = ctx.enter_context(tc.tile_pool(name="idx", bufs=16))
        cst_pool = ctx.enter_context(tc.tile_pool(name="cst", bufs=1))
        dram_pool = ctx.enter_context(tc.tile_pool(name="dram", bufs=2, space="DRAM"))
        ps_g = ctx.enter_context(tc.tile_pool(name="psg", bufs=2, space="PSUM"))
        ps_u = ctx.enter_context(tc.tile_pool(name="psu", bufs=2, space="PSUM"))
        ps_o = ctx.enter_context(tc.tile_pool(name="pso", bufs=2, space="PSUM"))
        ps_r = ctx.enter_context(tc.tile_pool(name="psr", bufs=2, space="PSUM"))

        # ---- constants ----
        ioc = cst_pool.tile([P, C], F32, tag="ioc")
        nc.sync.dma_start(ioc[:], iotaC[:])
        iot2 = cst_pool.tile([P, MT], FP16, tag="iot2")
        nc.sync.dma_start(iot2[:], iotaT2[:])
        tri = cst_pool.tile([P, P], F32R, tag="tri")
        nc.sync.dma_start(tri[:], triu[:])
        one = cst_pool.tile([P, P], F32R, tag="one")
        nc.sync.dma_start(one[:], ones[:])
        gwct = gwc_pool.tile([P, KD, E], F32, tag="gwc")
        nc.sync.dma_start(gwct[:], bass.AP(tensor=gwT, offset=0,
                                           ap=[[E, P], [E * P, KD], [1, E]]))

        # ---- x tiles for the router: four 1MB quarter loads, 2 bufs ----
        def load_xq(q):
            t = xts_pool.tile([P, KD, 256], F32R, tag="xts", bufs=2,
                              name="xq")
            nc.sync.dma_start(t[:], bass.AP(tensor=xT, offset=q * 256,
                                            ap=[[T, P], [T * P, KD],
                                                [1, 256]]))
            return t

        # ---- router: logits per token chunk (fp32, x-chunk stationary) ----
        cw_tiles, mask_tiles = [], []
        xq = load_xq(0)
        for mt in range(MT):
            if mt % 2 == 0 and mt > 0:
                xq = load_xq(mt // 2)
            mo = mt % 2
            pl = ps_r.tile([P, E], F32, tag="psr", name=f"pl{mt}")
            for k in range(KD):
                nc.tensor.matmul(pl[:],
                                 xq[:, k, mo * P:(mo + 1) * P].bitcast(F32),
                                 gwct[:, k, :],
                                 start=(k == 0), stop=(k == KD - 1))
            m1 = rt_pool.tile([P, 1], F32, tag="m1")
            nc.vector.reduce_max(m1[:], pl[:], axis=AX.X)
            nm1 = rt_pool.tile([P, 1], F32, tag="nm1")
            nc.vector.tensor_scalar(nm1[:], m1[:], -1.0, None, op0=OP.mult)
            t1 = rt_pool.tile([P, E], F32, tag="t1")
            nc.vector.tensor_scalar(t1[:], pl[:], m1[:], None, op0=OP.is_ge)
            lm = rt_pool.tile([P, E], F32, tag="lm")
            nc.vector.scalar_tensor_tensor(lm[:], t1[:], -1e30, pl[:],
                                           op0=OP.mult, op1=OP.add)
            m2 = rt_pool.tile([P, 1], F32, tag="m2")
            nc.vector.reduce_max(m2[:], lm[:], axis=AX.X)
            el = rt_pool.tile([P, E], F32, tag="el")
            nc.scalar.activation(el[:], pl[:], AF.Exp, bias=nm1[:])
            ssum = rt_pool.tile([P, 1], F32, tag="ssum")
            nc.vector.reduce_sum(ssum[:], el[:], axis=AX.X)
            el1 = rt_pool.tile([P, 1], F32, tag="el1")
            nc.vector.reduce_max(el1[:], el[:], axis=AX.X)
            el2 = rt_pool.tile([P, 1], F32, tag="el2")
            nc.scalar.activation(el2[:], m2[:], AF.Exp, bias=nm1[:])
            den = rt_pool.tile([P, 1], F32, tag="den")
            nc.vector.tensor_tensor(den[:], el1[:], el2[:], op=OP.add)
            nc.vector.scalar_tensor_tensor(den[:], ssum[:], 1e-8, den[:],
                                           op0=OP.mult, op1=OP.add)
            rec = rt_pool.tile([P, 1], F32, tag="rec")
            nc.vector.reciprocal(rec[:], den[:])
            msk = rt_pool.tile([P, E], F32R, tag="msk", bufs=MT, name=f"msk{mt}")
            nc.vector.tensor_scalar(msk[:], pl[:], m2[:], None, op0=OP.is_ge)
            cwu = rt_pool.tile([P, E], F32, tag="cwu")
            nc.vector.tensor_tensor(cwu[:], msk[:], el[:], op=OP.mult)
            cw = rt_pool.tile([P, E], F32, tag="cw", bufs=MT, name=f"cw{mt}")
            nc.vector.tensor_scalar(cw[:], cwu[:], rec[:], None, op0=OP.mult)
            cw_tiles.append(cw)
            mask_tiles.append(msk)

        # ---- exclusive prefix counts R[mt] [P, E] over token order ----
        # R[mt] = cumsum(cols of msk[<mt]) + tri*msk[mt]; cum kept in SBUF.
        r_tiles = []
        cum_prev = None
        for mt in range(MT):
            pr = ps_r.tile([P, E], F32, tag="psr", name=f"pr{mt}")
            nc.tensor.matmul(pr[:], tri[:], mask_tiles[mt][:],
                             start=True, stop=True)
            rsb = rt_pool.tile([P, E], F32, tag="rsb", bufs=MT, name=f"rsb{mt}")
            if cum_prev is None:
                nc.vector.tensor_copy(rsb[:], pr[:])
            else:
                nc.vector.tensor_tensor(rsb[:], pr[:], cum_prev[:], op=OP.add)
            r_tiles.append(rsb)
            if mt < MT - 1:
                pc = ps_r.tile([P, E], F32, tag="psr", name=f"pc{mt}")
                nc.tensor.matmul(pc[:], one[:], mask_tiles[mt][:],
                                 start=True, stop=True)
                cum = rt_pool.tile([P, E], F32, tag="cum", bufs=2, name=f"cum{mt}")
                if cum_prev is None:
                    nc.vector.tensor_copy(cum[:], pc[:])
                else:
                    nc.vector.tensor_tensor(cum[:], pc[:], cum_prev[:], op=OP.add)
                cum_prev = cum

        # ---- expert weights (bf16, packed 938): one DMA per tensor ----
        def load_w(j):
            wg = wgu_pool.tile([P, KD, GU2], BF16, tag="wgu", bufs=2,
                               name=f"wg{j}")
            nc.sync.dma_start(wg[:, :, 0:I], bass.AP(
                tensor=wgu, offset=j * D * GU2,
                ap=[[GU2, P], [GU2 * P, KD], [1, I]]))
            nc.sync.dma_start(wg[:, :, I:GU2], bass.AP(
                tensor=wgu, offset=j * D * GU2 + I,
                ap=[[GU2, P], [GU2 * P, KD], [1, I]]))
            wd = wdn_pool.tile([P, MI, D], BF16, tag="wdn", bufs=2,
                               name=f"wd{j}")
            nc.sync.dma_start(wd[:], bass.AP(
                tensor=wdn, offset=j * IP * D,
                ap=[[D, P], [D * P, MI], [1, D]]))
            return wg, wd

        # ---- prep(e): slot ids (+1 coded, -1 empty) + gatings, staged ----
        tstage = dram_pool.tile([NE, C], I16, tag="tstage", name="tstage")
        cstage = dram_pool.tile([NE, C], F32, tag="cstage", name="cstage")

        def prep(e):
            ex = e - 1
            sels = []
            for mt in range(MT):
                rk = rt_pool.tile([P, 1], F32, tag="rk")
                nc.vector.tensor_tensor(rk[:], r_tiles[mt][:, ex:ex + 1],
                                        mask_tiles[mt][:, ex:ex + 1], op=OP.mult)
                rks = rt_pool.tile([P, 1], F32, tag="rks")
                nc.vector.scalar_tensor_tensor(rks[:], mask_tiles[mt][:, ex:ex + 1],
                                               -1.0, rk[:], op0=OP.add, op1=OP.add)
                sl = sel_pool.tile([P, C], FP16, tag="sel", name=f"sel{e}_{mt}")
                nc.vector.tensor_scalar(sl[:], ioc[:], rks[:], None, op0=OP.is_equal)
                sels.append(sl)
            rhs2s = []
            for mt in range(MT):
                r2 = rhs2_pool.tile([P, 2], FP16, tag="rhs2", name=f"r2_{e}_{mt}")
                nc.vector.tensor_copy(r2[:, 0:1], iot2[:, mt:mt + 1])
                nc.vector.tensor_copy(r2[:, 1:2], cw_tiles[mt][:, ex:ex + 1])
                rhs2s.append(r2)
            ptc = ps_r.tile([2, C], F32, tag="psr", name=f"ptc{e}")
            for mt in range(MT):
                nc.tensor.matmul(ptc[:], rhs2s[mt][:, 0:2], sels[mt][:],
                                 start=(mt == 0), stop=(mt == MT - 1))
            # ids were coded as tokid+1; empty slots sum to 0 -> -1 sentinel
            pcs = idx_pool.tile([2, C], F32, tag="pcs", bufs=2, name=f"pcs{e}")
            nc.vector.tensor_copy(pcs[:], ptc[0:2, :])
            tok16i = idx_pool.tile([1, C], I16, tag="tok16i", bufs=2, name=f"tok16i{e}")
            nc.vector.tensor_scalar(tok16i[:], pcs[0:1, :], -1.0, 0.0,
                                    op0=OP.add, op1=OP.max)
            nc.sync.dma_start(
                bass.AP(tensor=tstage.tensor, offset=tstage.offset + e * C,
                        ap=[[1, C]]), tok16i[:])
            nc.sync.dma_start(
                bass.AP(tensor=cstage.tensor, offset=cstage.offset + e * C,
                        ap=[[1, C]]), pcs[1:2, :])
            idxw = idx_pool.tile([P, C // 16], I16, tag="idxw", bufs=NE,
                                 name=f"idxw{e}")
            for g in range(8):
                nc.sync.dma_start(
                    idxw[16 * g:16 * (g + 1), :],
                    bass.AP(tensor=tstage.tensor, offset=tstage.offset + e * C,
                            ap=[[1, 16], [16, C // 16]]))
            cwsT = idx_pool.tile([P, CB], F32, tag="cwsT", bufs=NE,
                                 name=f"cwsT{e}")
            nc.sync.dma_start(
                cwsT[:],
                bass.AP(tensor=cstage.tensor, offset=cstage.offset + e * C,
                        ap=[[1, P], [P, CB]]))
            return idxw, cwsT

        def gather(e, idxw):
            xet = xet_pool.tile([P, KD, C], BF16, tag="xet", name=f"xet{e}")
            nc.gpsimd.dma_gather(xet[:], xRbf[:], idxw[:], num_idxs=C,
                                 num_idxs_reg=C, elem_size=D, transpose=True)
            return xet

        # stage1 swiglu over token width w; rhs(k) -> moving AP [128, w]
        def swiglu_block(wg, rhs, w):
            ats = []
            for m in range(MI):
                mw = P if m < MI - 1 else MW_LAST
                pg = ps_g.tile([P, 512], F32, tag="psg", name=f"pg{m}")
                for k in range(KD):
                    nc.tensor.matmul(pg[0:mw, 0:w],
                                     wg[:, k, m * P:m * P + mw],
                                     rhs(k),
                                     start=(k == 0), stop=(k == KD - 1))
                pu = ps_u.tile([P, 512], F32, tag="psu", name=f"pu{m}")
                for k in range(KD):
                    nc.tensor.matmul(pu[0:mw, 0:w],
                                     wg[:, k, I + m * P:I + m * P + mw],
                                     rhs(k),
                                     start=(k == 0), stop=(k == KD - 1))
                st = st_pool.tile([P, 512], F32, tag="st", name=f"st{m}")
                nc.scalar.activation(st[0:mw, 0:w], pg[0:mw, 0:w], AF.Silu)
                at = at_pool.tile([P, 512], BF16, tag="at", name=f"at{m}")
                nc.vector.tensor_tensor(at[0:mw, 0:w], st[0:mw, 0:w],
                                        pu[0:mw, 0:w], op=OP.mult)
                ats.append(at)
            return ats

        # ---- shared expert (dense, bf16, direct output writes) ----
        if _DBG < 2:
            zt = ot_pool.tile([P, 512], BF16, tag="ot", name="z0")
            nc.vector.memset(zt[:], 0.0)
            for r in range(T // P):
                for nd in range(2):
                    nc.sync.dma_start(out[r * P:(r + 1) * P,
                                          nd * 512:(nd + 1) * 512], zt[:])
        wg0, wd0 = (load_w(0) if _DBG >= 2 else (None, None))
        xtbfs = []
        if _DBG >= 2:
            for k in range(KD):
                t = xtbf_pool.tile([P, T], BF16, tag="xtbf", name=f"xtbf{k}")
                nc.sync.dma_start(t[:], xTbf[k * P:(k + 1) * P, :])
                xtbfs.append(t)

        # weights for expert 1 post before the prep DMA burst (which waits
        # on router results) so the sync queue doesn't head-of-line block it
        weights = {}
        if _DBG >= 3:
            weights[1] = load_w(1)

        # ---- all-expert prep upfront; gathers prefetch GATHER_AHEAD ----
        preps = {}
        xets = {}
        if _DBG >= 3:
            for e in range(1, NE):
                preps[e] = prep(e)
            for e in range(1, 1 + GATHER_AHEAD):
                xets[e] = gather(e, preps[e][0])

        for ci, (off, w) in enumerate(SH_CHUNKS if _DBG >= 2 else []):
            ats = swiglu_block(wg0, lambda k: xtbfs[k][:, off:off + w], w)
            for sub in range(w // P):
                for nd in range(2):
                    po = ps_o.tile([P, 512], F32, tag="pso",
                                   name=f"spo{ci}_{sub}_{nd}")
                    for m in range(MI):
                        mw = P if m < MI - 1 else MW_LAST
                        nc.tensor.matmul(po[:],
                                         ats[m][0:mw, sub * P:(sub + 1) * P],
                                         wd0[0:mw, m, nd * 512:(nd + 1) * 512],
                                         start=(m == 0), stop=(m == MI - 1))
                    ot = ot_pool.tile([P, 512], BF16, tag="ot",
                                      name=f"so{ci}_{sub}_{nd}")
                    nc.vector.tensor_copy(ot[:], po[:])
                    rows = off + sub * P
                    nc.sync.dma_start(out[rows:rows + P,
                                          nd * 512:(nd + 1) * 512], ot[:])

        # ---- routed experts ----
        for e in range(1, NE if _DBG >= 3 else 1):
            idxw, cwsT = preps.pop(e)
            xet = xets.pop(e)
            if e + GATHER_AHEAD < NE:
                xets[e + GATHER_AHEAD] = gather(e + GATHER_AHEAD,
                                                preps[e + GATHER_AHEAD][0])
            if e + 1 < NE:
                weights[e + 1] = load_w(e + 1)
            if _DBG < 4:
                continue
            wg, wd = weights.pop(e)
            wE = NE_W[e - 1]
            ats = swiglu_block(wg, lambda k: xet[:, k, 0:wE], wE)

            for cb in range(CB):
                lo = cb * P
                cbw = min(P, wE - lo)
                if cbw <= 0:
                    break
                ysb = y_pool.tile([P, 1, D], BF16, tag="y", name=f"y{e}_{cb}")
                for nd in range(2):
                    po = ps_o.tile([P, 512], F32, tag="pso",
                                   name=f"po{e}_{cb}_{nd}")
                    for m in range(MI):
                        mw = P if m < MI - 1 else MW_LAST
                        nc.tensor.matmul(po[0:cbw, :],
                                         ats[m][0:mw, lo:lo + cbw],
                                         wd[0:mw, m, nd * 512:(nd + 1) * 512],
                                         start=(m == 0), stop=(m == MI - 1))
                    nc.vector.tensor_scalar(ysb[0:cbw, 0, nd * 512:(nd + 1) * 512],
                                            po[0:cbw, :], cwsT[0:cbw, cb:cb + 1],
                                            None, op0=OP.mult)
                nc.gpsimd.dma_scatter_add(out[:], ysb[:],
                                          idxw[:, cb * 8:cb * 8 + (cbw + 15) // 16],
                                          num_idxs=cbw, num_idxs_reg=cbw,
                                          elem_size=D)

    nc.compile()
    return nc


_NC_CACHE = None


def _get_nc():
    global _NC_CACHE
    if _NC_CACHE is None:
        _NC_CACHE = build_moe()
    return _NC_CACHE


def _prep_weights(gate_weight, shared_gate_up, shared_down,
                  experts_gate_up, experts_down):
    bf = ml_dtypes.bfloat16
    wgu = np.empty((NE, D, GU2), bf)
    wgu[0, :, 0:I] = shared_gate_up[0:I].T.astype(bf)
    wgu[0, :, I:GU2] = shared_gate_up[I:2 * I].T.astype(bf)
    for e in range(E):
        wgu[e + 1, :, 0:I] = experts_gate_up[e, 0:I].T.astype(bf)
        wgu[e + 1, :, I:GU2] = experts_gate_up[e, I:2 * I].T.astype(bf)
    wdn = np.zeros((NE, IP, D), bf)
    wdn[0, 0:I] = shared_down.T.astype(bf)
    for e in range(E):
        wdn[e + 1, 0:I] = experts_down[e].T.astype(bf)
    gwT = np.ascontiguousarray(gate_weight.T.astype(np.float32))
    return gwT, np.ascontiguousarray(wgu), np.ascontiguousarray(wdn)


def _consts():
    iota_t2 = (np.arange(P, dtype=np.float32)[:, None] +
               P * np.arange(MT, dtype=np.float32)[None, :]) + 1.0
    return {
        "triu": np.triu(np.ones((P, P), np.float32), 1),
        "ones": np.ones((P, P), np.float32),
        "iotaC": np.broadcast_to(np.arange(C, dtype=np.float32), (P, C)).copy(),
        "iotaT2": iota_t2.astype(np.float16),
    }


def make_in_maps(hidden_states, gate_weight, shared_gate_up, shared_down,
                 experts_gate_up, experts_down):
    bf = ml_dtypes.bfloat16
    hidden_states = np.asarray(hidden_states, dtype=np.float32)
    x = hidden_states.reshape(B * S, D)
    gwT, wgu, wdn = _prep_weights(
        np.asarray(gate_weight, np.float32),
        np.asarray(shared_gate_up, np.float32),
        np.asarray(shared_down, np.float32),
        np.asarray(experts_gate_up, np.float32),
        np.asarray(experts_down, np.float32))
    consts = _consts()
    in_maps = []
    for c in range(N_CORES):
        xs = np.ascontiguousarray(x[c * T:(c + 1) * T])
        xsT = np.ascontiguousarray(xs.T)
        in_maps.append({
            "xT": xsT,
            "xTbf": xsT.astype(bf),
            "xRbf": xs.astype(bf),
            "gwT": gwT, "wgu": wgu, "wdn": wdn, **consts,
        })
    return in_maps


def kernel(hidden_states, gate_weight, shared_gate_up, shared_down,
           experts_gate_up, experts_down):
    in_maps = make_in_maps(hidden_states, gate_weight, shared_gate_up,
                           shared_down, experts_gate_up, experts_down)
    nc = _get_nc()
    res = run_bass_kernel_spmd(nc, in_maps, core_ids=list(range(N_CORES)))
    out = np.concatenate([np.asarray(res.results[c]["out"], dtype=np.float32)
                          for c in range(N_CORES)], axis=0)
    return out.reshape(B, S, D)


# revision 18
# speedup vs baseline: 1.0304x; 1.0304x over previous
"""MoE layer v4: token-data-parallel, bf16 FFN, transpose-gather.

Per core (1024 tokens): router computes top-2 combine weights in fp32
(x-chunk-stationary matmuls on quarter-T x tiles); per-expert exclusive
ranks via a running-cumsum of triangular/ones matmuls. All-expert prep
runs immediately after the router (during the shared expert): per expert
a 0/1 slot-selection matrix in fp16 and ONE flipped matmul chain ([128,2]
stationary: tokid+1 | gating) yields slot->(tokid+1) and slot gatings on
PSUM partitions 0-1; ids get -1 so empty slots hold -1 (trailing) and the
scatter skips them. ids/gatings roundtrip DRAM to become gather indices
(idx layout [128, C/16], single strided DMA) and a [128,3] gating layout.
dma_gather(transpose=True) pulls 384 selected token rows from the bf16
token table directly as X_e^T; gathers run 2-3 experts ahead. SwiGLU runs
in bf16 on a per-expert compute width NE_W[e] = max-load + margin <= 320
slots with packed 938-wide weights; outputs are scaled by slot gatings
and dma_scatter_add'ed per capacity chunk (128,128,NE_W-256) into the
bf16 output, skipping empty slots. The shared expert runs dense in bf16
over two 512-token chunks, writing output rows directly. Weight / x / gw
loads are batched into few 3D-AP DMAs to minimize serialized per-
instruction DMA issue cost on the sync engine.
"""

import os
import numpy as np
from contextlib import ExitStack

_DBG = 4  # debug build levels retained for bisection; 4 = full kernel

import ml_dtypes

import concourse.bass as bass
import concourse.mybir as mybir
import concourse.tile as tile
from concourse import bacc
from concourse.bass_utils import run_bass_kernel_spmd

B, S, D = 4, 2048, 1024
E = 8
I = 938
GU2 = 2 * I          # 1876 packed gate|up
NE = E + 1
N_CORES = 8
T = (B * S) // N_CORES   # 1024 tokens/core
C = 384                  # gather slot space (transpose gather needs %128)
CB = C // 128

# per-expert compute width: max load over cores (282 global) + margin
NE_W = [296, 292, 296, 304, 300, 296, 296, 296]

P = 128
KD = D // P              # 8 d-chunks
MT = T // P              # 8 token chunks
MI = (I + P - 1) // P    # 8 i-chunks (last = 42 rows)
MW_LAST = I - (MI - 1) * P   # 42
IP = MI * P              # 1024 (wdn dram padded rows)

F32 = mybir.dt.float32
F32R = mybir.dt.float32r
BF16 = mybir.dt.bfloat16
FP16 = mybir.dt.float16
I16 = mybir.dt.int16
AF = mybir.ActivationFunctionType
OP = mybir.AluOpType
AX = mybir.AxisListType

SH_CHUNKS = [(0, 512), (512, 512)]   # shared expert token chunks
GATHER_AHEAD = 3                     # experts of gather prefetch


def build_moe():
    nc = bacc.Bacc("TRN2", target_bir_lowering=False, debug=False,
                   enable_asserts=True, num_devices=N_CORES)
    xT = nc.dram_tensor("xT", [D, T], F32R, kind="ExternalInput")
    xTbf = nc.dram_tensor("xTbf", [D, T], BF16, kind="ExternalInput")
    xRbf = nc.dram_tensor("xRbf", [T, D], BF16, kind="ExternalInput")
    gwT = nc.dram_tensor("gwT", [D, E], F32, kind="ExternalInput")
    wgu = nc.dram_tensor("wgu", [NE, D, GU2], BF16, kind="ExternalInput")
    wdn = nc.dram_tensor("wdn", [NE, IP, D], BF16, kind="ExternalInput")
    triu = nc.dram_tensor("triu", [P, P], F32R, kind="ExternalInput")
    ones = nc.dram_tensor("ones", [P, P], F32R, kind="ExternalInput")
    iotaC = nc.dram_tensor("iotaC", [P, C], F32, kind="ExternalInput")
    iotaT2 = nc.dram_tensor("iotaT2", [P, MT], FP16, kind="ExternalInput")
    out = nc.dram_tensor("out", [T, D], BF16, kind="ExternalOutput")

    with tile.TileContext(nc) as tc, ExitStack() as ctx:
        xts_pool = ctx.enter_context(tc.tile_pool(name="xts", bufs=2))
        gwc_pool = ctx.enter_context(tc.tile_pool(name="gwc", bufs=KD))
        xtbf_pool = ctx.enter_context(tc.tile_pool(name="xtbf", bufs=KD))
        wgu_pool = ctx.enter_context(tc.tile_pool(name="wgu", bufs=16))
        wdn_pool = ctx.enter_context(tc.tile_pool(name="wdn", bufs=15))
        xet_pool = ctx.enter_context(tc.tile_pool(name="xet", bufs=3))
        at_pool = ctx.enter_context(tc.tile_pool(name="at", bufs=12))
        st_pool = ctx.enter_context(tc.tile_pool(name="st", bufs=4))
        ot_pool = ctx.enter_context(tc.tile_pool(name="ot", bufs=4))
        y_pool = ctx.enter_context(tc.tile_pool(name="y", bufs=3))
        sel_pool = ctx.enter_context(tc.tile_pool(name="sel", bufs=8))
        rt_pool = ctx.enter_context(tc.tile_pool(name="rt", bufs=4))
        rhs2_pool = ctx.enter_context(tc.tile_pool(name="rhs2", bufs=8))
        idx_pool = ctx.enter_context(tc.tile_pool(name="idx", bufs=16))
        cst_pool = ctx.enter_context(tc.tile_pool(name="cst", bufs=1))
        dram_pool = ctx.enter_context(tc.tile_pool(name="dram", bufs=2, space="DRAM"))
        ps_g = ctx.enter_context(tc.tile_pool(name="psg", bufs=2, space="PSUM"))
        ps_u = ctx.enter_context(tc.tile_pool(name="psu", bufs=2, space="PSUM"))
        ps_o = ctx.enter_context(tc.tile_pool(name="pso", bufs=2, space="PSUM"))
        ps_r = ctx.enter_context(tc.tile_pool(name="psr", bufs=2, space="PSUM"))

        # ---- constants ----
        ioc = cst_pool.tile([P, C], F32, tag="ioc")
        nc.sync.dma_start(ioc[:], iotaC[:])
        iot2 = cst_pool.tile([P, MT], FP16, tag="iot2")
        nc.sync.dma_start(iot2[:], iotaT2[:])
        tri = cst_pool.tile([P, P], F32R, tag="tri")
        nc.sync.dma_start(tri[:], triu[:])
        one = cst_pool.tile([P, P], F32R, tag="one")
        nc.sync.dma_start(one[:], ones[:])
        gwct = gwc_pool.tile([P, KD, E], F32, tag="gwc")
        nc.sync.dma_start(gwct[:], bass.AP(tensor=gwT, offset=0,
                                           ap=[[E, P], [E * P, KD], [1, E]]))

        # ---- x tiles for the router: four 1MB quarter loads, 2 bufs ----
        def load_xq(q):
            t = xts_pool.tile([P, KD, 256], F32R, tag="xts", bufs=2,
                              name="xq")
            nc.sync.dma_start(t[:], bass.AP(tensor=xT, offset=q * 256,
                                            ap=[[T, P], [T * P, KD],
                                                [1, 256]]))
            return t

        # ---- router: logits per token chunk (fp32, x-chunk stationary) ----
        cw_tiles, mask_tiles = [], []
        xq = load_xq(0)
        for mt in range(MT):
            if mt % 2 == 0 and mt > 0:
                xq = load_xq(mt // 2)
            mo = mt % 2
            pl = ps_r.tile([P, E], F32, tag="psr", name=f"pl{mt}")
            for k in range(KD):
                nc.tensor.matmul(pl[:],
                                 xq[:, k, mo * P:(mo + 1) * P].bitcast(F32),
                                 gwct[:, k, :],
                                 start=(k == 0), stop=(k == KD - 1))
            m1 = rt_pool.tile([P, 1], F32, tag="m1")
            nc.vector.reduce_max(m1[:], pl[:], axis=AX.X)
            nm1 = rt_pool.tile([P, 1], F32, tag="nm1")
            nc.vector.tensor_scalar(nm1[:], m1[:], -1.0, None, op0=OP.mult)
            t1 = rt_pool.tile([P, E], F32, tag="t1")
            nc.vector.tensor_scalar(t1[:], pl[:], m1[:], None, op0=OP.is_ge)
            lm = rt_pool.tile([P, E], F32, tag="lm")
            nc.vector.scalar_tensor_tensor(lm[:], t1[:], -1e30, pl[:],
                                           op0=OP.mult, op1=OP.add)
            m2 = rt_pool.tile([P, 1], F32, tag="m2")
            nc.vector.reduce_max(m2[:], lm[:], axis=AX.X)
            el = rt_pool.tile([P, E], F32, tag="el")
            nc.scalar.activation(el[:], pl[:], AF.Exp, bias=nm1[:])
            ssum = rt_pool.tile([P, 1], F32, tag="ssum")
            nc.vector.reduce_sum(ssum[:], el[:], axis=AX.X)
            el1 = rt_pool.tile([P, 1], F32, tag="el1")
            nc.vector.reduce_max(el1[:], el[:], axis=AX.X)
            el2 = rt_pool.tile([P, 1], F32, tag="el2")
            nc.scalar.activation(el2[:], m2[:], AF.Exp, bias=nm1[:])
            den = rt_pool.tile([P, 1], F32, tag="den")
            nc.vector.tensor_tensor(den[:], el1[:], el2[:], op=OP.add)
            nc.vector.scalar_tensor_tensor(den[:], ssum[:], 1e-8, den[:],
                                           op0=OP.mult, op1=OP.add)
            rec = rt_pool.tile([P, 1], F32, tag="rec")
            nc.vector.reciprocal(rec[:], den[:])
            msk = rt_pool.tile([P, E], F32R, tag="msk", bufs=MT, name=f"msk{mt}")
            nc.vector.tensor_scalar(msk[:], pl[:], m2[:], None, op0=OP.is_ge)
            cwu = rt_pool.tile([P, E], F32, tag="cwu")
            nc.vector.tensor_tensor(cwu[:], msk[:], el[:], op=OP.mult)
            cw = rt_pool.tile([P, E], F32, tag="cw", bufs=MT, name=f"cw{mt}")
            nc.vector.tensor_scalar(cw[:], cwu[:], rec[:], None, op0=OP.mult)
            cw_tiles.append(cw)
            mask_tiles.append(msk)

        # ---- exclusive prefix counts R[mt] [P, E] over token order ----
        # R[mt] = cumsum(cols of msk[<mt]) + tri*msk[mt]; cum kept in SBUF.
        r_tiles = []
        cum_prev = None
        for mt in range(MT):
            pr = ps_r.tile([P, E], F32, tag="psr", name=f"pr{mt}")
            nc.tensor.matmul(pr[:], tri[:], mask_tiles[mt][:],
                             start=True, stop=True)
            rsb = rt_pool.tile([P, E], F32, tag="rsb", bufs=MT, name=f"rsb{mt}")
            if cum_prev is None:
                nc.vector.tensor_copy(rsb[:], pr[:])
            else:
                nc.vector.tensor_tensor(rsb[:], pr[:], cum_prev[:], op=OP.add)
            r_tiles.append(rsb)
            if mt < MT - 1:
                pc = ps_r.tile([P, E], F32, tag="psr", name=f"pc{mt}")
                nc.tensor.matmul(pc[:], one[:], mask_tiles[mt][:],
                                 start=True, stop=True)
                cum = rt_pool.tile([P, E], F32, tag="cum", bufs=2, name=f"cum{mt}")
                if cum_prev is None:
                    nc.vector.tensor_copy(cum[:], pc[:])
                else:
                    nc.vector.tensor_tensor(cum[:], pc[:], cum_prev[:], op=OP.add)
                cum_prev = cum

        # ---- expert weights (bf16, packed 938): one DMA per tensor ----
        def load_w(j):
            wg = wgu_pool.tile([P, KD, GU2], BF16, tag="wgu", bufs=2,
                               name=f"wg{j}")
            nc.sync.dma_start(wg[:, :, 0:I], bass.AP(
                tensor=wgu, offset=j * D * GU2,
                ap=[[GU2, P], [GU2 * P, KD], [1, I]]))
            nc.sync.dma_start(wg[:, :, I:GU2], bass.AP(
                tensor=wgu, offset=j * D * GU2 + I,
                ap=[[GU2, P], [GU2 * P, KD], [1, I]]))
            wd = wdn_pool.tile([P, MI, D], BF16, tag="wdn", bufs=2,
                               name=f"wd{j}")
            nc.sync.dma_start(wd[:], bass.AP(
                tensor=wdn, offset=j * IP * D,
                ap=[[D, P], [D * P, MI], [1, D]]))
            return wg, wd

        # ---- prep(e): slot ids (+1 coded, -1 empty) + gatings, staged ----
        tstage = dram_pool.tile([NE, C], I16, tag="tstage", name="tstage")
        cstage = dram_pool.tile([NE, C], F32, tag="cstage", name="cstage")

        def prep(e):
            ex = e - 1
            sels = []
            for mt in range(MT):
                rk = rt_pool.tile([P, 1], F32, tag="rk")
                nc.vector.tensor_tensor(rk[:], r_tiles[mt][:, ex:ex + 1],
                                        mask_tiles[mt][:, ex:ex + 1], op=OP.mult)
                rks = rt_pool.tile([P, 1], F32, tag="rks")
                nc.vector.tensor_tensor(rks[:], rk[:],
                                        mask_tiles[mt][:, ex:ex + 1], op=OP.add)
                # iotaC holds slot+1: (rank+1)*msk == slot+1 selects; 0 never
                sl = sel_pool.tile([P, C], FP16, tag="sel", name=f"sel{e}_{mt}")
                nc.vector.tensor_scalar(sl[:], ioc[:], rks[:], None, op0=OP.is_equal)
                sels.append(sl)
            rhs2s = []
            for mt in range(MT):
                r2 = rhs2_pool.tile([P, 2], FP16, tag="rhs2", name=f"r2_{e}_{mt}")
                nc.vector.tensor_copy(r2[:, 0:1], iot2[:, mt:mt + 1])
                nc.vector.tensor_copy(r2[:, 1:2], cw_tiles[mt][:, ex:ex + 1])
                rhs2s.append(r2)
            ptc = ps_r.tile([2, C], F32, tag="psr", name=f"ptc{e}")
            for mt in range(MT):
                nc.tensor.matmul(ptc[:], rhs2s[mt][:, 0:2], sels[mt][:],
                                 start=(mt == 0), stop=(mt == MT - 1))
            # ids were coded as tokid+1; empty slots sum to 0 -> -1 sentinel
            pcs = idx_pool.tile([2, C], F32, tag="pcs", bufs=2, name=f"pcs{e}")
            nc.vector.tensor_copy(pcs[:], ptc[0:2, :])
            tok16i = idx_pool.tile([1, C], I16, tag="tok16i", bufs=2, name=f"tok16i{e}")
            nc.vector.tensor_scalar(tok16i[:], pcs[0:1, :], -1.0, 0.0,
                                    op0=OP.add, op1=OP.max)
            nc.sync.dma_start(
                bass.AP(tensor=tstage.tensor, offset=tstage.offset + e * C,
                        ap=[[1, C]]), tok16i[:])
            nc.sync.dma_start(
                bass.AP(tensor=cstage.tensor, offset=cstage.offset + e * C,
                        ap=[[1, C]]), pcs[1:2, :])
            idxw = idx_pool.tile([P, C // 16], I16, tag="idxw", bufs=NE,
                                 name=f"idxw{e}")
            for g in range(8):
                nc.sync.dma_start(
                    idxw[16 * g:16 * (g + 1), :],
                    bass.AP(tensor=tstage.tensor, offset=tstage.offset + e * C,
                            ap=[[1, 16], [16, C // 16]]))
            cwsT = idx_pool.tile([P, CB], F32, tag="cwsT", bufs=NE,
                                 name=f"cwsT{e}")
            nc.sync.dma_start(
                cwsT[:],
                bass.AP(tensor=cstage.tensor, offset=cstage.offset + e * C,
                        ap=[[1, P], [P, CB]]))
            return idxw, cwsT

        def gather(e, idxw):
            xet = xet_pool.tile([P, KD, C], BF16, tag="xet", name=f"xet{e}")
            nc.gpsimd.dma_gather(xet[:], xRbf[:], idxw[:], num_idxs=C,
                                 num_idxs_reg=C, elem_size=D, transpose=True)
            return xet

        # stage1 swiglu over token width w; rhs(k) -> moving AP [128, w]
        def swiglu_block(wg, rhs, w):
            ats = []
            for m in range(MI):
                mw = P if m < MI - 1 else MW_LAST
                pg = ps_g.tile([P, 512], F32, tag="psg", name=f"pg{m}")
                for k in range(KD):
                    nc.tensor.matmul(pg[0:mw, 0:w],
                                     wg[:, k, m * P:m * P + mw],
                                     rhs(k),
                                     start=(k == 0), stop=(k == KD - 1))
                pu = ps_u.tile([P, 512], F32, tag="psu", name=f"pu{m}")
                for k in range(KD):
                    nc.tensor.matmul(pu[0:mw, 0:w],
                                     wg[:, k, I + m * P:I + m * P + mw],
                                     rhs(k),
                                     start=(k == 0), stop=(k == KD - 1))
                st = st_pool.tile([P, 512], F32, tag="st", name=f"st{m}")
                nc.scalar.activation(st[0:mw, 0:w], pg[0:mw, 0:w], AF.Silu)
                at = at_pool.tile([P, 512], BF16, tag="at", name=f"at{m}")
                nc.vector.tensor_tensor(at[0:mw, 0:w], st[0:mw, 0:w],
                                        pu[0:mw, 0:w], op=OP.mult)
                ats.append(at)
            return ats

        # ---- shared expert (dense, bf16, direct output writes) ----
        if _DBG < 2:
            zt = ot_pool.tile([P, 512], BF16, tag="ot", name="z0")
            nc.vector.memset(zt[:], 0.0)
            for r in range(T // P):
                for nd in range(2):
                    nc.sync.dma_start(out[r * P:(r + 1) * P,
                                          nd * 512:(nd + 1) * 512], zt[:])
        wg0, wd0 = (load_w(0) if _DBG >= 2 else (None, None))
        xtbfs = []
        if _DBG >= 2:
            for k in range(KD):
                t = xtbf_pool.tile([P, T], BF16, tag="xtbf", name=f"xtbf{k}")
                nc.sync.dma_start(t[:], xTbf[k * P:(k + 1) * P, :])
                xtbfs.append(t)

        # weights for expert 1 post before the prep DMA burst (which waits
        # on router results) so the sync queue doesn't head-of-line block it
        weights = {}
        preps = {}
        xets = {}
        if _DBG >= 3:
            weights[1] = load_w(1)

        # preps interleave with shared-expert stages so their vector-engine
        # ops don't head-of-line block shared's activation multiplies
        def prep_group(lo, hi, gathers):
            if _DBG < 3:
                return
            for e in range(lo, hi + 1):
                preps[e] = prep(e)
            for e in gathers:
                xets[e] = gather(e, preps[e][0])

        def shared_s1(off, w):
            return swiglu_block(wg0, lambda k: xtbfs[k][:, off:off + w], w)

        def shared_down(ats, ci, off, w):
            for sub in range(w // P):
                for nd in range(2):
                    po = ps_o.tile([P, 512], F32, tag="pso",
                                   name=f"spo{ci}_{sub}_{nd}")
                    for m in range(MI):
                        mw = P if m < MI - 1 else MW_LAST
                        nc.tensor.matmul(po[:],
                                         ats[m][0:mw, sub * P:(sub + 1) * P],
                                         wd0[0:mw, m, nd * 512:(nd + 1) * 512],
                                         start=(m == 0), stop=(m == MI - 1))
                    ot = ot_pool.tile([P, 512], BF16, tag="ot",
                                      name=f"so{ci}_{sub}_{nd}")
                    nc.scalar.activation(ot[:], po[:], AF.Copy)
                    rows = off + sub * P
                    nc.sync.dma_start(out[rows:rows + P,
                                          nd * 512:(nd + 1) * 512], ot[:])

        if _DBG >= 2:
            ats0 = shared_s1(0, 512)
            prep_group(1, 2, [1])
            shared_down(ats0, 0, 0, 512)
            prep_group(3, 4, [2, 3])
            ats1 = shared_s1(512, 512)
            prep_group(5, 6, [])
            shared_down(ats1, 1, 512, 512)
            prep_group(7, 8, [])
        elif _DBG >= 3:
            prep_group(1, NE - 1, [1, 2, 3])

        # ---- routed experts ----
        for e in range(1, NE if _DBG >= 3 else 1):
            idxw, cwsT = preps.pop(e)
            xet = xets.pop(e)
            if e + GATHER_AHEAD < NE:
                xets[e + GATHER_AHEAD] = gather(e + GATHER_AHEAD,
                                                preps[e + GATHER_AHEAD][0])
            if e + 1 < NE:
                weights[e + 1] = load_w(e + 1)
            if _DBG < 4:
                continue
            wg, wd = weights.pop(e)
            wE = NE_W[e - 1]
            ats = swiglu_block(wg, lambda k: xet[:, k, 0:wE], wE)

            for cb in range(CB):
                lo = cb * P
                cbw = min(P, wE - lo)
                if cbw <= 0:
                    break
                ysb = y_pool.tile([P, 1, D], BF16, tag="y", name=f"y{e}_{cb}")
                for nd in range(2):
                    po = ps_o.tile([P, 512], F32, tag="pso",
                                   name=f"po{e}_{cb}_{nd}")
                    for m in range(MI):
                        mw = P if m < MI - 1 else MW_LAST
                        nc.tensor.matmul(po[0:cbw, :],
                                         ats[m][0:mw, lo:lo + cbw],
                                         wd[0:mw, m, nd * 512:(nd + 1) * 512],
                                         start=(m == 0), stop=(m == MI - 1))
                    nc.scalar.activation(ysb[0:cbw, 0, nd * 512:(nd + 1) * 512],
                                         po[0:cbw, :], AF.Copy,
                                         scale=cwsT[0:cbw, cb:cb + 1])
                nc.gpsimd.dma_scatter_add(out[:], ysb[:],
                                          idxw[:, cb * 8:cb * 8 + (cbw + 15) // 16],
                                          num_idxs=cbw, num_idxs_reg=cbw,
                                          elem_size=D)

    nc.compile()
    return nc


_NC_CACHE = None


def _get_nc():
    global _NC_CACHE
    if _NC_CACHE is None:
        _NC_CACHE = build_moe()
    return _NC_CACHE


def _prep_weights(gate_weight, shared_gate_up, shared_down,
                  experts_gate_up, experts_down):
    bf = ml_dtypes.bfloat16
    wgu = np.empty((NE, D, GU2), bf)
    wgu[0, :, 0:I] = shared_gate_up[0:I].T.astype(bf)
    wgu[0, :, I:GU2] = shared_gate_up[I:2 * I].T.astype(bf)
    for e in range(E):
        wgu[e + 1, :, 0:I] = experts_gate_up[e, 0:I].T.astype(bf)
        wgu[e + 1, :, I:GU2] = experts_gate_up[e, I:2 * I].T.astype(bf)
    wdn = np.zeros((NE, IP, D), bf)
    wdn[0, 0:I] = shared_down.T.astype(bf)
    for e in range(E):
        wdn[e + 1, 0:I] = experts_down[e].T.astype(bf)
    gwT = np.ascontiguousarray(gate_weight.T.astype(np.float32))
    return gwT, np.ascontiguousarray(wgu), np.ascontiguousarray(wdn)


def _consts():
    iota_t2 = (np.arange(P, dtype=np.float32)[:, None] +
               P * np.arange(MT, dtype=np.float32)[None, :]) + 1.0
    return {
        "triu": np.triu(np.ones((P, P), np.float32), 1),
        "ones": np.ones((P, P), np.float32),
        "iotaC": np.broadcast_to(np.arange(C, dtype=np.float32) + 1.0,
                                 (P, C)).copy(),
        "iotaT2": iota_t2.astype(np.float16),
    }


def make_in_maps(hidden_states, gate_weight, shared_gate_up, shared_down,
                 experts_gate_up, experts_down):
    bf = ml_dtypes.bfloat16
    hidden_states = np.asarray(hidden_states, dtype=np.float32)
    x = hidden_states.reshape(B * S, D)
    gwT, wgu, wdn = _prep_weights(
        np.asarray(gate_weight, np.float32),
        np.asarray(shared_gate_up, np.float32),
        np.asarray(shared_down, np.float32),
        np.asarray(experts_gate_up, np.float32),
        np.asarray(experts_down, np.float32))
    consts = _consts()
    in_maps = []
    for c in range(N_CORES):
        xs = np.ascontiguousarray(x[c * T:(c + 1) * T])
        xsT = np.ascontiguousarray(xs.T)
        in_maps.append({
            "xT": xsT,
            "xTbf": xsT.astype(bf),
            "xRbf": xs.astype(bf),
            "gwT": gwT, "wgu": wgu, "wdn": wdn, **consts,
        })
    return in_maps


def kernel(hidden_states, gate_weight, shared_gate_up, shared_down,
           experts_gate_up, experts_down):
    in_maps = make_in_maps(hidden_states, gate_weight, shared_gate_up,
                           shared_down, experts_gate_up, experts_down)
    nc = _get_nc()
    res = run_bass_kernel_spmd(nc, in_maps, core_ids=list(range(N_CORES)))
    out = np.concatenate([np.asarray(res.results[c]["out"], dtype=np.float32)
                          for c in range(N_CORES)], axis=0)
    return out.reshape(B, S, D)
